# revision 4
# baseline (speedup 1.0000x reference)
"""Trainium2 Bass kernel for the KAN-style Fourier-feature layer.

Computes out[b,t,h] = sum_{f,c} basis(x)[b,t,f,c] * W[f,c,h] + sum_f b[f,h]
where basis = [1, sin x, cos x, sin 2x, cos 2x, ..., sin 5x, cos 5x].

Strategy (8-way data-parallel over batch*seq = 131072 tokens):
  - Host: range-reduce x to [-pi, pi] (fp64, exact mod identity for all
    harmonics since sin(k(x-2pi n)) = sin(kx)), fold the Chebyshev
    trig->monomial basis change into W in fp64.
  - Device per core: ACT computes sin, |x|, cos (= sin(pi/2-|x|)), c^2, c^4
    (Squares); DVE/GPSIMD compute the remaining monomials {s*c, c^3, s*c^2,
    s*c^3, s*c^4, c^5}; PE contracts with K=32 matmuls, 4 token-groups run
    concurrently via row+col tile_position packing; PSUM accumulates over
    the 10 monomials; bias folds into the PSUM->SBUF eviction.
"""

import sys

sys.path.insert(0, "/opt/trn_rl_repo")

import numpy as np

import concourse.bacc as bacc
import concourse.mybir as mybir
from concourse import tile
from concourse.bass_utils import run_bass_kernel_spmd

AF = mybir.ActivationFunctionType
F32 = mybir.dt.float32

NCORES = 8
B, T, F, H = 8, 16384, 32, 64
TOKPC = (B * T) // NCORES          # tokens per core = 16384
NGRP = 4                           # token groups stacked on partitions
GTOK = TOKPC // NGRP               # tokens per group = 4096
NBLK = 4                           # blocks per core
BLKCOL = GTOK // NBLK              # free-dim columns per block = 1024
HALF = 512                         # matmul moving free dim (fp32 max)
NHALF = BLKCOL // HALF             # halves per block = 2
NJ = 10                            # non-constant monomial basis functions

# Monomial order (matmul accumulation order j = 0..9):
#   s, c, s*c, c^2, s*c^2, c^3, s*c^3, c^4, s*c^4, c^5
# Trig basis (reference order [1, s1, c1, s2, c2, s3, c3, s4, c4, s5, c5])
# expressed in monomials [const, s, c, sc, c2, sc2, c3, sc3, c4, sc4, c5]:
_A = np.zeros((11, 11), dtype=np.float64)
_A[0, 0] = 1.0                       # 1
_A[1, 1] = 1.0                       # sin x = s
_A[2, 2] = 1.0                       # cos x = c
_A[3, 3] = 2.0                       # sin 2x = 2 s c
_A[4, 0], _A[4, 4] = -1.0, 2.0       # cos 2x = 2c^2 - 1
_A[5, 1], _A[5, 5] = -1.0, 4.0       # sin 3x = 4 s c^2 - s
_A[6, 2], _A[6, 6] = -3.0, 4.0       # cos 3x = 4c^3 - 3c
_A[7, 3], _A[7, 7] = -4.0, 8.0       # sin 4x = 8 s c^3 - 4 s c
_A[8, 0], _A[8, 4], _A[8, 8] = 1.0, -8.0, 8.0    # cos 4x = 8c^4 - 8c^2 + 1
_A[9, 1], _A[9, 5], _A[9, 9] = 1.0, -12.0, 16.0  # sin 5x = 16 s c^4 - 12 s c^2 + s
_A[10, 2], _A[10, 6], _A[10, 10] = 5.0, -20.0, 16.0  # cos 5x = 16c^5 - 20c^3 + 5c

_PROG = None


def _build_program():
    nc = bacc.Bacc(None, target_bir_lowering=False)
    x_d = nc.declare_dram_parameter("x", [128, GTOK], F32, isOutput=False)
    w_d = nc.declare_dram_parameter("w", [128, NJ * 128], F32, isOutput=False)
    bias_d = nc.declare_dram_parameter("bias", [128, 1], F32, isOutput=False)
    out_d = nc.declare_dram_parameter("out", [128, TOKPC * H // 128], F32, isOutput=True)

    with tile.TileContext(nc) as tc:
        with (
            tc.tile_pool(name="const", bufs=1) as cpool,
            tc.tile_pool(name="wpool", bufs=1) as wpool,
            tc.tile_pool(name="xpool", bufs=2) as xpool,
            tc.tile_pool(name="fpool", bufs=2) as fpool,
            tc.tile_pool(name="opool", bufs=2) as opool,
            tc.tile_pool(name="psum", bufs=4, space="PSUM") as ppool,
        ):
            halfpi = cpool.tile([128, 1], F32, tag="halfpi")
            nc.vector.memset(halfpi[:], float(np.pi / 2))
            bias_sb = cpool.tile([128, 1], F32, tag="bias")
            nc.sync.dma_start(out=bias_sb[:], in_=bias_d[:])
            w_sb = wpool.tile([128, NJ, 128], F32, tag="w")
            nc.sync.dma_start(
                out=w_sb[:], in_=w_d[:].rearrange("p (j m) -> p j m", j=NJ)
            )

            for blk in range(NBLK):
                cs = slice(blk * BLKCOL, (blk + 1) * BLKCOL)
                xt = xpool.tile([128, BLKCOL], F32, tag="x")
                nc.sync.dma_start(out=xt[:], in_=x_d[:, cs])

                s = fpool.tile([128, BLKCOL], F32, tag="s")
                ab = fpool.tile([128, BLKCOL], F32, tag="ab")
                c = fpool.tile([128, BLKCOL], F32, tag="c")
                c2 = fpool.tile([128, BLKCOL], F32, tag="c2")
                c4 = fpool.tile([128, BLKCOL], F32, tag="c4")
                sc = fpool.tile([128, BLKCOL], F32, tag="sc")
                c3 = fpool.tile([128, BLKCOL], F32, tag="c3")
                sc2 = fpool.tile([128, BLKCOL], F32, tag="sc2")
                sc3 = fpool.tile([128, BLKCOL], F32, tag="sc3")
                sc4 = fpool.tile([128, BLKCOL], F32, tag="sc4")
                c5 = fpool.tile([128, BLKCOL], F32, tag="c5")

                nc.scalar.activation(s[:], xt[:], AF.Sin)
                nc.scalar.activation(ab[:], xt[:], AF.Abs)
                nc.scalar.activation(c[:], ab[:], AF.Sin, bias=halfpi[:], scale=-1.0)
                nc.scalar.activation(c2[:], c[:], AF.Square)
                nc.scalar.activation(c4[:], c2[:], AF.Square)

                nc.vector.tensor_mul(sc[:], s[:], c[:])
                nc.vector.tensor_mul(c3[:], c[:], c2[:])
                nc.vector.tensor_mul(sc2[:], s[:], c2[:])
                nc.vector.tensor_mul(sc3[:], sc[:], c2[:])
                nc.gpsimd.tensor_mul(sc4[:], s[:], c4[:])
                nc.gpsimd.tensor_mul(c5[:], c[:], c4[:])

                forder = [s, c, sc, c2, sc2, c3, sc3, c4, sc4, c5]

                out_sb = opool.tile([128, 2 * BLKCOL], F32, tag="o")
                for half in range(NHALF):
                    hs = slice(half * HALF, (half + 1) * HALF)
                    ps = [
                        ppool.tile([128, HALF], F32, name=f"ps{a}", tag=f"ps{a}")
                        for a in range(2)
                    ]
                    for j in range(NJ):
                        for g in range(NGRP):
                            a, bcol = g // 2, g % 2
                            nc.tensor.matmul(
                                ps[a][64 * bcol : 64 * bcol + 64, :],
                                w_sb[32 * g : 32 * g + 32, j, 64 * bcol : 64 * bcol + 64],
                                forder[j][32 * g : 32 * g + 32, hs],
                                start=(j == 0),
                                stop=(j == NJ - 1),
                                tile_position=(32 * g, 64 * bcol),
                            )
                    for a in range(2):
                        dst = out_sb[:, (2 * half + a) * HALF : (2 * half + a + 1) * HALF]
                        if (half + a) % 2 == 0:
                            nc.scalar.activation(
                                dst, ps[a][:], AF.Identity, bias=bias_sb[:], scale=1.0
                            )
                        else:
                            nc.vector.tensor_scalar_add(dst, ps[a][:], bias_sb[:])
                nc.sync.dma_start(
                    out=out_d[:, blk * 2 * BLKCOL : (blk + 1) * 2 * BLKCOL],
                    in_=out_sb[:],
                )

    nc.compile()
    return nc


def _get_program():
    global _PROG
    if _PROG is None:
        _PROG = _build_program()
    return _PROG


def _prep_inputs(x, W, b):
    """Host-side: shard + layout x, fold basis transform into W."""
    x = np.asarray(x)
    W64 = np.asarray(W, dtype=np.float64)
    b64 = np.asarray(b, dtype=np.float64)

    # W2[f, m, h] = sum_i A[i, m] * W[f, i, h]
    W2 = np.einsum("im,fih->fmh", _A, W64)
    bias = W2[:, 0, :].sum(axis=0) + b64.sum(axis=0)          # [H]
    bias_col = np.tile(bias, 2).astype(np.float32)[:, None]    # [128, 1]
    bias_col = np.ascontiguousarray(bias_col)

    # per-j matmul weights, replicated 4x along K-groups and 2x along H
    w_list = [np.tile(W2[:, 1 + j, :], (NGRP, 2)) for j in range(NJ)]  # [128,128]
    w_flat = (
        np.stack(w_list, axis=0).transpose(1, 0, 2).reshape(128, NJ * 128)
    ).astype(np.float32)
    w_flat = np.ascontiguousarray(w_flat)

    xt = x.reshape(B * T, F).astype(np.float64)
    xr = np.mod(xt + np.pi, 2.0 * np.pi) - np.pi               # [-pi, pi)
    xr = xr.astype(np.float32)

    xs = []
    for cid in range(NCORES):
        xc = xr[cid * TOKPC : (cid + 1) * TOKPC]               # [16384, 32]
        xc = xc.reshape(NGRP, GTOK, F).transpose(0, 2, 1).reshape(128, GTOK)
        xs.append(np.ascontiguousarray(xc))
    return xs, w_flat, bias_col


def _decode_out(outc):
    """[128, 8192] device layout -> [TOKPC, H]."""
    arr = outc.reshape(2, H, NBLK, NHALF, 2, HALF)   # [bcol, h, blk, half, a, cc]
    arr = arr.transpose(4, 0, 2, 3, 5, 1)            # [a, bcol, blk, half, cc, h]
    return arr.reshape(TOKPC, H)


LAST_RESULT = None


def kernel(x, W, b, trace=False, tmpdir=None):
    nc = _get_program()
    xs, w_flat, bias_col = _prep_inputs(x, W, b)
    in_maps = [
        {"x": xs[cid], "w": w_flat, "bias": bias_col} for cid in range(NCORES)
    ]
    res = run_bass_kernel_spmd(
        nc, in_maps, list(range(NCORES)), trace=trace, tmpdir=tmpdir
    )
    global LAST_RESULT
    LAST_RESULT = res
    out = np.empty((B * T, H), dtype=np.float32)
    for cid in range(NCORES):
        out[cid * TOKPC : (cid + 1) * TOKPC] = _decode_out(res.results[cid]["out"])
    return out.reshape(B, T, H)


# revision 7
# speedup vs baseline: 1.2231x; 1.2231x over previous
"""Trainium2 Bass kernel for the KAN-style Fourier-feature layer.

Computes out[b,t,h] = sum_{f,c} basis(x)[b,t,f,c] * W[f,c,h] + sum_f b[f,h]
where basis = [1, sin x, cos x, sin 2x, cos 2x, ..., sin 5x, cos 5x].

Strategy (8-way data-parallel over batch*seq = 131072 tokens):
  - Host: range-reduce x to [-pi, pi] (fp64, exact mod identity for all
    harmonics since sin(k(x-2pi n)) = sin(kx)), fold the Chebyshev
    trig->monomial basis change into W in fp64.
  - Device per core: ACT computes sin, |x|, cos (= sin(pi/2-|x|)), c^2, c^4
    (Squares); DVE/GPSIMD compute the remaining monomials {s*c, c^3, s*c^2,
    s*c^3, s*c^4, c^5}; PE contracts with K=32 matmuls, 4 token-groups run
    concurrently via row+col tile_position packing; PSUM accumulates over
    the 10 monomials; bias folds into the PSUM->SBUF eviction.
"""

import sys

sys.path.insert(0, "/opt/trn_rl_repo")

import numpy as np

import concourse.bacc as bacc
import concourse.mybir as mybir
from concourse import tile
from concourse.bass_utils import run_bass_kernel_spmd

AF = mybir.ActivationFunctionType
F32 = mybir.dt.float32

NCORES = 8
B, T, F, H = 8, 16384, 32, 64
TOKPC = (B * T) // NCORES          # tokens per core = 16384
NGRP = 4                           # token groups stacked on partitions
GTOK = TOKPC // NGRP               # tokens per group = 4096
NBLK = 4                           # blocks per core
BLKCOL = GTOK // NBLK              # free-dim columns per block = 1024
HALF = 512                         # matmul moving free dim (fp32 max)
NHALF = BLKCOL // HALF             # halves per block = 2
NJ = 10                            # non-constant monomial basis functions

# Monomial order (matmul accumulation order j = 0..9):
#   s, c, s*c, c^2, s*c^2, c^3, s*c^3, c^4, s*c^4, c^5
# Trig basis (reference order [1, s1, c1, s2, c2, s3, c3, s4, c4, s5, c5])
# expressed in monomials [const, s, c, sc, c2, sc2, c3, sc3, c4, sc4, c5]:
_A = np.zeros((11, 11), dtype=np.float64)
_A[0, 0] = 1.0                       # 1
_A[1, 1] = 1.0                       # sin x = s
_A[2, 2] = 1.0                       # cos x = c
_A[3, 3] = 2.0                       # sin 2x = 2 s c
_A[4, 0], _A[4, 4] = -1.0, 2.0       # cos 2x = 2c^2 - 1
_A[5, 1], _A[5, 5] = -1.0, 4.0       # sin 3x = 4 s c^2 - s
_A[6, 2], _A[6, 6] = -3.0, 4.0       # cos 3x = 4c^3 - 3c
_A[7, 3], _A[7, 7] = -4.0, 8.0       # sin 4x = 8 s c^3 - 4 s c
_A[8, 0], _A[8, 4], _A[8, 8] = 1.0, -8.0, 8.0    # cos 4x = 8c^4 - 8c^2 + 1
_A[9, 1], _A[9, 5], _A[9, 9] = 1.0, -12.0, 16.0  # sin 5x = 16 s c^4 - 12 s c^2 + s
_A[10, 2], _A[10, 6], _A[10, 10] = 5.0, -20.0, 16.0  # cos 5x = 16c^5 - 20c^3 + 5c

_PROG = None


def _build_program():
    nc = bacc.Bacc(None, target_bir_lowering=False)
    x_d = nc.declare_dram_parameter("x", [128, GTOK], F32, isOutput=False)
    w_d = nc.declare_dram_parameter("w", [128, NJ * 128], F32, isOutput=False)
    bias_d = nc.declare_dram_parameter("bias", [128, 1], F32, isOutput=False)
    out_d = nc.declare_dram_parameter("out", [128, TOKPC * H // 128], F32, isOutput=True)

    with tile.TileContext(nc) as tc:
        with (
            tc.tile_pool(name="const", bufs=1) as cpool,
            tc.tile_pool(name="wpool", bufs=1) as wpool,
            tc.tile_pool(name="xpool", bufs=2) as xpool,
            tc.tile_pool(name="fpool", bufs=2) as fpool,
            tc.tile_pool(name="opool", bufs=2) as opool,
            tc.tile_pool(name="psum", bufs=2, space="PSUM") as ppool,
        ):
            halfpi = cpool.tile([128, 1], F32, tag="halfpi")
            nc.vector.memset(halfpi[:], float(np.pi / 2))
            bias_sb = cpool.tile([128, 1], F32, tag="bias")
            nc.sync.dma_start(out=bias_sb[:], in_=bias_d[:])
            w_sb = wpool.tile([128, NJ, 128], F32, tag="w")
            nc.sync.dma_start(
                out=w_sb[:], in_=w_d[:].rearrange("p (j m) -> p j m", j=NJ)
            )

            for blk in range(NBLK):
                cs = slice(blk * BLKCOL, (blk + 1) * BLKCOL)
                xt = xpool.tile([128, BLKCOL], F32, tag="x")
                nc.sync.dma_start(out=xt[:], in_=x_d[:, cs])

                s = fpool.tile([128, BLKCOL], F32, tag="s")
                ab = fpool.tile([128, BLKCOL], F32, tag="ab")
                c = fpool.tile([128, BLKCOL], F32, tag="c")
                c2 = fpool.tile([128, BLKCOL], F32, tag="c2")
                c4 = fpool.tile([128, BLKCOL], F32, tag="c4")
                sc = fpool.tile([128, BLKCOL], F32, tag="sc")
                c3 = fpool.tile([128, BLKCOL], F32, tag="c3")
                sc2 = fpool.tile([128, BLKCOL], F32, tag="sc2")
                sc3 = fpool.tile([128, BLKCOL], F32, tag="sc3")
                sc4 = fpool.tile([128, BLKCOL], F32, tag="sc4")
                c5 = fpool.tile([128, BLKCOL], F32, tag="c5")

                nc.scalar.activation(s[:], xt[:], AF.Sin)
                nc.scalar.activation(ab[:], xt[:], AF.Abs)
                nc.scalar.activation(c[:], ab[:], AF.Sin, bias=halfpi[:], scale=-1.0)
                nc.scalar.activation(c2[:], c[:], AF.Square)
                nc.scalar.activation(c4[:], c2[:], AF.Square)

                nc.vector.tensor_mul(sc[:], s[:], c[:])
                nc.vector.tensor_mul(c3[:], c[:], c2[:])
                nc.vector.tensor_mul(sc2[:], s[:], c2[:])
                nc.vector.tensor_mul(sc3[:], sc[:], c2[:])
                nc.gpsimd.tensor_mul(sc4[:], s[:], c4[:])
                nc.gpsimd.tensor_mul(c5[:], c[:], c4[:])

                forder = [s, c, sc, c2, sc2, c3, sc3, c4, sc4, c5]

                out_sb = opool.tile([128, 2 * BLKCOL], F32, tag="o")
                # 8 concurrent MMs per j: 4 row-groups x both col-halves of
                # the PE array (half 1 uses the complementary col-group so
                # all 16 subarrays stay busy).
                ps = {
                    (half, a): ppool.tile(
                        [128, HALF], F32, name=f"ps{half}{a}", tag=f"ps{half}{a}"
                    )
                    for half in range(NHALF)
                    for a in range(2)
                }
                for j in range(NJ):
                    for g in range(NGRP):
                        a = g // 2
                        for half in range(NHALF):
                            bcol = (g + half) % 2
                            hs = slice(half * HALF, (half + 1) * HALF)
                            nc.tensor.matmul(
                                ps[(half, a)][64 * bcol : 64 * bcol + 64, :],
                                w_sb[32 * g : 32 * g + 32, j, 64 * bcol : 64 * bcol + 64],
                                forder[j][32 * g : 32 * g + 32, hs],
                                start=(j == 0),
                                stop=(j == NJ - 1),
                                tile_position=(32 * g, 64 * bcol),
                            )
                for half in range(NHALF):
                    for a in range(2):
                        dst = out_sb[:, (2 * half + a) * HALF : (2 * half + a + 1) * HALF]
                        if (half + a) % 2 == 0:
                            nc.scalar.activation(
                                dst,
                                ps[(half, a)][:],
                                AF.Identity,
                                bias=bias_sb[:],
                                scale=1.0,
                            )
                        else:
                            nc.vector.tensor_scalar_add(
                                dst, ps[(half, a)][:], bias_sb[:]
                            )
                nc.sync.dma_start(
                    out=out_d[:, blk * 2 * BLKCOL : (blk + 1) * 2 * BLKCOL],
                    in_=out_sb[:],
                )

    nc.compile()
    return nc


def _get_program():
    global _PROG
    if _PROG is None:
        _PROG = _build_program()
    return _PROG


def _prep_inputs(x, W, b):
    """Host-side: shard + layout x, fold basis transform into W."""
    x = np.asarray(x)
    W64 = np.asarray(W, dtype=np.float64)
    b64 = np.asarray(b, dtype=np.float64)

    # W2[f, m, h] = sum_i A[i, m] * W[f, i, h]
    W2 = np.einsum("im,fih->fmh", _A, W64)
    bias = W2[:, 0, :].sum(axis=0) + b64.sum(axis=0)          # [H]
    bias_col = np.tile(bias, 2).astype(np.float32)[:, None]    # [128, 1]
    bias_col = np.ascontiguousarray(bias_col)

    # per-j matmul weights, replicated 4x along K-groups and 2x along H
    w_list = [np.tile(W2[:, 1 + j, :], (NGRP, 2)) for j in range(NJ)]  # [128,128]
    w_flat = (
        np.stack(w_list, axis=0).transpose(1, 0, 2).reshape(128, NJ * 128)
    ).astype(np.float32)
    w_flat = np.ascontiguousarray(w_flat)

    xt = x.reshape(B * T, F).astype(np.float64)
    xr = np.mod(xt + np.pi, 2.0 * np.pi) - np.pi               # [-pi, pi)
    xr = xr.astype(np.float32)

    xs = []
    for cid in range(NCORES):
        xc = xr[cid * TOKPC : (cid + 1) * TOKPC]               # [16384, 32]
        xc = xc.reshape(NGRP, GTOK, F).transpose(0, 2, 1).reshape(128, GTOK)
        xs.append(np.ascontiguousarray(xc))
    return xs, w_flat, bias_col


def _decode_out(outc):
    """[128, 8192] device layout -> [TOKPC, H].

    Rows are 64*bcol + h; for half 0 group g sits at bcol = g%2, for half 1
    at bcol = (g+1)%2 (complementary PE col-group packing)."""
    arr = outc.reshape(2, H, NBLK, NHALF, 2, HALF)   # [bcol, h, blk, half, a, cc]
    h0 = arr[:, :, :, 0]                             # [b, h, blk, a, cc], g = 2a+b
    h1 = arr[::-1, :, :, 1]                          # b-index == g%2 after flip
    both = np.stack([h0, h1], axis=3)                # [b, h, blk, half, a, cc]
    both = both.transpose(4, 0, 2, 3, 5, 1)          # [a, b, blk, half, cc, h]
    return both.reshape(TOKPC, H)


LAST_RESULT = None


def kernel(x, W, b, trace=False, tmpdir=None):
    nc = _get_program()
    xs, w_flat, bias_col = _prep_inputs(x, W, b)
    in_maps = [
        {"x": xs[cid], "w": w_flat, "bias": bias_col} for cid in range(NCORES)
    ]
    res = run_bass_kernel_spmd(
        nc, in_maps, list(range(NCORES)), trace=trace, tmpdir=tmpdir
    )
    global LAST_RESULT
    LAST_RESULT = res
    out = np.empty((B * T, H), dtype=np.float32)
    for cid in range(NCORES):
        out[cid * TOKPC : (cid + 1) * TOKPC] = _decode_out(res.results[cid]["out"])
    return out.reshape(B, T, H)


# revision 10
# speedup vs baseline: 1.3414x; 1.0967x over previous
"""Trainium2 Bass kernel for the KAN-style Fourier-feature layer.

Computes out[b,t,h] = sum_{f,c} basis(x)[b,t,f,c] * W[f,c,h] + sum_f b[f,h]
where basis = [1, sin x, cos x, sin 2x, cos 2x, ..., sin 5x, cos 5x].

Strategy (8-way data-parallel over batch*seq = 131072 tokens):
  - Host: range-reduce x to [-pi, pi] (fp64, exact mod identity for all
    harmonics since sin(k(x-2pi n)) = sin(kx)), fold the Chebyshev
    trig->monomial basis change into W in fp64.
  - Device per core: ACT computes sin, |x|, cos (= sin(pi/2-|x|)), c^2, c^4
    (Squares); DVE/GPSIMD compute the remaining monomials {s*c, c^3, s*c^2,
    s*c^3, s*c^4, c^5}; PE contracts with K=32 matmuls, 4 token-groups run
    concurrently via row+col tile_position packing; PSUM accumulates over
    the 10 monomials; bias folds into the PSUM->SBUF eviction.
"""

import sys

sys.path.insert(0, "/opt/trn_rl_repo")

import numpy as np

import concourse.bacc as bacc
import concourse.mybir as mybir
from concourse import tile
from concourse.bass_utils import run_bass_kernel_spmd

AF = mybir.ActivationFunctionType
F32 = mybir.dt.float32

NCORES = 8
B, T, F, H = 8, 16384, 32, 64
TOKPC = (B * T) // NCORES          # tokens per core = 16384
NGRP = 4                           # token groups stacked on partitions
GTOK = TOKPC // NGRP               # tokens per group = 4096
NBLK = 4                           # blocks per core
BLKCOL = GTOK // NBLK              # free-dim columns per block = 1024
HALF = 512                         # matmul moving free dim (fp32 max)
NHALF = BLKCOL // HALF             # halves per block = 2
NJ = 10                            # non-constant monomial basis functions

# Monomial order (matmul accumulation order j = 0..9):
#   s, c, s*c, c^2, s*c^2, c^3, s*c^3, c^4, s*c^4, c^5
# Trig basis (reference order [1, s1, c1, s2, c2, s3, c3, s4, c4, s5, c5])
# expressed in monomials [const, s, c, sc, c2, sc2, c3, sc3, c4, sc4, c5]:
_A = np.zeros((11, 11), dtype=np.float64)
_A[0, 0] = 1.0                       # 1
_A[1, 1] = 1.0                       # sin x = s
_A[2, 2] = 1.0                       # cos x = c
_A[3, 3] = 2.0                       # sin 2x = 2 s c
_A[4, 0], _A[4, 4] = -1.0, 2.0       # cos 2x = 2c^2 - 1
_A[5, 1], _A[5, 5] = -1.0, 4.0       # sin 3x = 4 s c^2 - s
_A[6, 2], _A[6, 6] = -3.0, 4.0       # cos 3x = 4c^3 - 3c
_A[7, 3], _A[7, 7] = -4.0, 8.0       # sin 4x = 8 s c^3 - 4 s c
_A[8, 0], _A[8, 4], _A[8, 8] = 1.0, -8.0, 8.0    # cos 4x = 8c^4 - 8c^2 + 1
_A[9, 1], _A[9, 5], _A[9, 9] = 1.0, -12.0, 16.0  # sin 5x = 16 s c^4 - 12 s c^2 + s
_A[10, 2], _A[10, 6], _A[10, 10] = 5.0, -20.0, 16.0  # cos 5x = 16c^5 - 20c^3 + 5c

_PROG = None


def _build_program():
    nc = bacc.Bacc(None, target_bir_lowering=False)
    x_d = nc.declare_dram_parameter("x", [128, GTOK], F32, isOutput=False)
    w_d = nc.declare_dram_parameter("w", [128, NJ * 128], F32, isOutput=False)
    bias_d = nc.declare_dram_parameter("bias", [128, 1], F32, isOutput=False)
    out_d = nc.declare_dram_parameter("out", [128, TOKPC * H // 128], F32, isOutput=True)

    with tile.TileContext(nc) as tc:
        with (
            tc.tile_pool(name="const", bufs=1) as cpool,
            tc.tile_pool(name="wpool", bufs=1) as wpool,
            tc.tile_pool(name="xpool", bufs=2) as xpool,
            tc.tile_pool(name="fpool", bufs=3) as fpool,
            tc.tile_pool(name="opool", bufs=2) as opool,
            tc.tile_pool(name="psum", bufs=2, space="PSUM") as ppool,
        ):
            halfpi = cpool.tile([128, 1], F32, tag="halfpi")
            nc.vector.memset(halfpi[:], float(np.pi / 2))
            bias_sb = cpool.tile([128, 1], F32, tag="bias")
            nc.sync.dma_start(out=bias_sb[:], in_=bias_d[:])
            w_sb = wpool.tile([128, NJ, 128], F32, tag="w")
            nc.sync.dma_start(
                out=w_sb[:], in_=w_d[:].rearrange("p (j m) -> p j m", j=NJ)
            )

            for blk in range(NBLK):
                cs = slice(blk * BLKCOL, (blk + 1) * BLKCOL)
                xt = xpool.tile([128, BLKCOL], F32, tag="x")
                nc.sync.dma_start(out=xt[:], in_=x_d[:, cs])

                s = fpool.tile([128, BLKCOL], F32, tag="s")
                ab = fpool.tile([128, BLKCOL], F32, tag="ab")
                c = fpool.tile([128, BLKCOL], F32, tag="c")
                c2 = fpool.tile([128, BLKCOL], F32, tag="c2")
                c4 = fpool.tile([128, BLKCOL], F32, tag="c4")
                sc = fpool.tile([128, BLKCOL], F32, tag="sc")
                c3 = fpool.tile([128, BLKCOL], F32, tag="c3")
                sc2 = fpool.tile([128, BLKCOL], F32, tag="sc2")
                sc3 = fpool.tile([128, BLKCOL], F32, tag="sc3")
                sc4 = fpool.tile([128, BLKCOL], F32, tag="sc4")
                c5 = fpool.tile([128, BLKCOL], F32, tag="c5")

                nc.scalar.activation(s[:], xt[:], AF.Sin)
                nc.scalar.activation(ab[:], xt[:], AF.Abs)
                nc.scalar.activation(c[:], ab[:], AF.Sin, bias=halfpi[:], scale=-1.0)
                nc.scalar.activation(c2[:], c[:], AF.Square)
                nc.scalar.activation(c4[:], c2[:], AF.Square)

                nc.vector.tensor_mul(sc[:], s[:], c[:])
                nc.vector.tensor_mul(c3[:], c[:], c2[:])
                nc.vector.tensor_mul(sc2[:], s[:], c2[:])
                # alternate the 4th product between DVE and GPSIMD to balance
                eng = nc.vector if blk % 2 == 0 else nc.gpsimd
                eng.tensor_mul(sc3[:], sc[:], c2[:])
                nc.gpsimd.tensor_mul(sc4[:], s[:], c4[:])
                nc.gpsimd.tensor_mul(c5[:], c[:], c4[:])

                forder = [s, c, sc, c2, sc2, c3, sc3, c4, sc4, c5]

                out_sb = opool.tile([128, 2 * BLKCOL], F32, tag="o")
                # 8 concurrent MMs per j: 4 row-groups x both col-halves of
                # the PE array (half 1 uses the complementary col-group so
                # all 16 subarrays stay busy).
                ps = {
                    (half, a): ppool.tile(
                        [128, HALF], F32, name=f"ps{half}{a}", tag=f"ps{half}{a}"
                    )
                    for half in range(NHALF)
                    for a in range(2)
                }
                for j in range(NJ):
                    for g in range(NGRP):
                        a = g // 2
                        for half in range(NHALF):
                            bcol = (g + half) % 2
                            hs = slice(half * HALF, (half + 1) * HALF)
                            nc.tensor.matmul(
                                ps[(half, a)][64 * bcol : 64 * bcol + 64, :],
                                w_sb[32 * g : 32 * g + 32, j, 64 * bcol : 64 * bcol + 64],
                                forder[j][32 * g : 32 * g + 32, hs],
                                start=(j == 0),
                                stop=(j == NJ - 1),
                                tile_position=(32 * g, 64 * bcol),
                            )
                for half in range(NHALF):
                    for a in range(2):
                        dst = out_sb[:, (2 * half + a) * HALF : (2 * half + a + 1) * HALF]
                        nc.scalar.activation(
                            dst,
                            ps[(half, a)][:],
                            AF.Identity,
                            bias=bias_sb[:],
                            scale=1.0,
                        )
                nc.sync.dma_start(
                    out=out_d[:, blk * 2 * BLKCOL : (blk + 1) * 2 * BLKCOL],
                    in_=out_sb[:],
                )

    nc.compile()
    return nc


def _get_program():
    global _PROG
    if _PROG is None:
        _PROG = _build_program()
    return _PROG


def _prep_inputs(x, W, b):
    """Host-side: shard + layout x, fold basis transform into W."""
    x = np.asarray(x)
    W64 = np.asarray(W, dtype=np.float64)
    b64 = np.asarray(b, dtype=np.float64)

    # W2[f, m, h] = sum_i A[i, m] * W[f, i, h]
    W2 = np.einsum("im,fih->fmh", _A, W64)
    bias = W2[:, 0, :].sum(axis=0) + b64.sum(axis=0)          # [H]
    bias_col = np.tile(bias, 2).astype(np.float32)[:, None]    # [128, 1]
    bias_col = np.ascontiguousarray(bias_col)

    # per-j matmul weights, replicated 4x along K-groups and 2x along H
    w_list = [np.tile(W2[:, 1 + j, :], (NGRP, 2)) for j in range(NJ)]  # [128,128]
    w_flat = (
        np.stack(w_list, axis=0).transpose(1, 0, 2).reshape(128, NJ * 128)
    ).astype(np.float32)
    w_flat = np.ascontiguousarray(w_flat)

    xt = x.reshape(B * T, F).astype(np.float64)
    xr = np.mod(xt + np.pi, 2.0 * np.pi) - np.pi               # [-pi, pi)
    xr = xr.astype(np.float32)

    xs = []
    for cid in range(NCORES):
        xc = xr[cid * TOKPC : (cid + 1) * TOKPC]               # [16384, 32]
        xc = xc.reshape(NGRP, GTOK, F).transpose(0, 2, 1).reshape(128, GTOK)
        xs.append(np.ascontiguousarray(xc))
    return xs, w_flat, bias_col


def _decode_out(outc):
    """[128, 8192] device layout -> [TOKPC, H].

    Rows are 64*bcol + h; for half 0 group g sits at bcol = g%2, for half 1
    at bcol = (g+1)%2 (complementary PE col-group packing)."""
    arr = outc.reshape(2, H, NBLK, NHALF, 2, HALF)   # [bcol, h, blk, half, a, cc]
    h0 = arr[:, :, :, 0]                             # [b, h, blk, a, cc], g = 2a+b
    h1 = arr[::-1, :, :, 1]                          # b-index == g%2 after flip
    both = np.stack([h0, h1], axis=3)                # [b, h, blk, half, a, cc]
    both = both.transpose(4, 0, 2, 3, 5, 1)          # [a, b, blk, half, cc, h]
    return both.reshape(TOKPC, H)


LAST_RESULT = None


def kernel(x, W, b, trace=False, tmpdir=None):
    nc = _get_program()
    xs, w_flat, bias_col = _prep_inputs(x, W, b)
    in_maps = [
        {"x": xs[cid], "w": w_flat, "bias": bias_col} for cid in range(NCORES)
    ]
    res = run_bass_kernel_spmd(
        nc, in_maps, list(range(NCORES)), trace=trace, tmpdir=tmpdir
    )
    global LAST_RESULT
    LAST_RESULT = res
    out = np.empty((B * T, H), dtype=np.float32)
    for cid in range(NCORES):
        out[cid * TOKPC : (cid + 1) * TOKPC] = _decode_out(res.results[cid]["out"])
    return out.reshape(B, T, H)


# revision 12
# speedup vs baseline: 1.4215x; 1.0597x over previous
"""Trainium2 Bass kernel for the KAN-style Fourier-feature layer.

Computes out[b,t,h] = sum_{f,c} basis(x)[b,t,f,c] * W[f,c,h] + sum_f b[f,h]
where basis = [1, sin x, cos x, sin 2x, cos 2x, ..., sin 5x, cos 5x].

Strategy (8-way data-parallel over batch*seq = 131072 tokens):
  - Host: range-reduce x to [-pi, pi] (fp64, exact mod identity for all
    harmonics since sin(k(x-2pi n)) = sin(kx)), fold the Chebyshev
    trig->monomial basis change into W in fp64.
  - Device per core: ACT computes sin, |x|, cos (= sin(pi/2-|x|)), c^2, c^4
    (Squares); DVE/GPSIMD compute the remaining monomials {s*c, c^3, s*c^2,
    s*c^3, s*c^4, c^5}; PE contracts with K=32 matmuls, 4 token-groups run
    concurrently via row+col tile_position packing; PSUM accumulates over
    the 10 monomials; bias folds into the PSUM->SBUF eviction.
"""

import sys

sys.path.insert(0, "/opt/trn_rl_repo")

import numpy as np

import concourse.bacc as bacc
import concourse.mybir as mybir
from concourse import tile
from concourse.bass_utils import run_bass_kernel_spmd

AF = mybir.ActivationFunctionType
F32 = mybir.dt.float32

NCORES = 8
B, T, F, H = 8, 16384, 32, 64
TOKPC = (B * T) // NCORES          # tokens per core = 16384
NGRP = 4                           # token groups stacked on partitions
GTOK = TOKPC // NGRP               # tokens per group = 4096
NBLK = 4                           # blocks per core
BLKCOL = GTOK // NBLK              # free-dim columns per block = 1024
HALF = 512                         # matmul moving free dim (fp32 max)
NHALF = BLKCOL // HALF             # halves per block = 2
NJ = 10                            # non-constant monomial basis functions

# Monomial order (matmul accumulation order j = 0..9):
#   s, c, s*c, c^2, s*c^2, c^3, s*c^3, c^4, s*c^4, c^5
# Trig basis (reference order [1, s1, c1, s2, c2, s3, c3, s4, c4, s5, c5])
# expressed in monomials [const, s, c, sc, c2, sc2, c3, sc3, c4, sc4, c5]:
_A = np.zeros((11, 11), dtype=np.float64)
_A[0, 0] = 1.0                       # 1
_A[1, 1] = 1.0                       # sin x = s
_A[2, 2] = 1.0                       # cos x = c
_A[3, 3] = 2.0                       # sin 2x = 2 s c
_A[4, 0], _A[4, 4] = -1.0, 2.0       # cos 2x = 2c^2 - 1
_A[5, 1], _A[5, 5] = -1.0, 4.0       # sin 3x = 4 s c^2 - s
_A[6, 2], _A[6, 6] = -3.0, 4.0       # cos 3x = 4c^3 - 3c
_A[7, 3], _A[7, 7] = -4.0, 8.0       # sin 4x = 8 s c^3 - 4 s c
_A[8, 0], _A[8, 4], _A[8, 8] = 1.0, -8.0, 8.0    # cos 4x = 8c^4 - 8c^2 + 1
_A[9, 1], _A[9, 5], _A[9, 9] = 1.0, -12.0, 16.0  # sin 5x = 16 s c^4 - 12 s c^2 + s
_A[10, 2], _A[10, 6], _A[10, 10] = 5.0, -20.0, 16.0  # cos 5x = 16c^5 - 20c^3 + 5c

_PROG = None


def _build_program():
    nc = bacc.Bacc(None, target_bir_lowering=False)
    x_d = nc.declare_dram_parameter("x", [128, GTOK], F32, isOutput=False)
    w_d = nc.declare_dram_parameter("w", [128, NJ * 128], F32, isOutput=False)
    bias_d = nc.declare_dram_parameter("bias", [128, 1], F32, isOutput=False)
    out_d = nc.declare_dram_parameter("out", [128, TOKPC * H // 128], F32, isOutput=True)

    with tile.TileContext(nc) as tc:
        with (
            tc.tile_pool(name="const", bufs=1) as cpool,
            tc.tile_pool(name="wpool", bufs=1) as wpool,
            tc.tile_pool(name="xpool", bufs=2) as xpool,
            tc.tile_pool(name="fpool", bufs=3) as fpool,
            tc.tile_pool(name="opool", bufs=2) as opool,
            tc.tile_pool(name="psum", bufs=2, space="PSUM") as ppool,
        ):
            halfpi = cpool.tile([128, 1], F32, tag="halfpi")
            nc.vector.memset(halfpi[:], float(np.pi / 2))
            bias_sb = cpool.tile([128, 1], F32, tag="bias")
            nc.sync.dma_start(out=bias_sb[:], in_=bias_d[:])
            w_sb = wpool.tile([128, NJ, 128], F32, tag="w")
            nc.sync.dma_start(
                out=w_sb[:], in_=w_d[:].rearrange("p (j m) -> p j m", j=NJ)
            )

            for blk in range(NBLK):
                cs = slice(blk * BLKCOL, (blk + 1) * BLKCOL)
                xt = xpool.tile([128, BLKCOL], F32, tag="x")
                nc.sync.dma_start(out=xt[:], in_=x_d[:, cs])

                s = fpool.tile([128, BLKCOL], F32, tag="s")
                ab = fpool.tile([128, BLKCOL], F32, tag="ab")
                c = fpool.tile([128, BLKCOL], F32, tag="c")
                c2 = fpool.tile([128, BLKCOL], F32, tag="c2")
                c4 = fpool.tile([128, BLKCOL], F32, tag="c4")
                sc = fpool.tile([128, BLKCOL], F32, tag="sc")
                c3 = fpool.tile([128, BLKCOL], F32, tag="c3")
                sc2 = fpool.tile([128, BLKCOL], F32, tag="sc2")
                sc3 = fpool.tile([128, BLKCOL], F32, tag="sc3")
                sc4 = fpool.tile([128, BLKCOL], F32, tag="sc4")
                c5 = fpool.tile([128, BLKCOL], F32, tag="c5")

                nc.scalar.activation(s[:], xt[:], AF.Sin)
                nc.scalar.activation(ab[:], xt[:], AF.Abs)
                nc.scalar.activation(c[:], ab[:], AF.Sin, bias=halfpi[:], scale=-1.0)
                nc.scalar.activation(c2[:], c[:], AF.Square)
                nc.scalar.activation(c4[:], c2[:], AF.Square)

                nc.vector.tensor_mul(sc[:], s[:], c[:])
                nc.vector.tensor_mul(c3[:], c[:], c2[:])
                nc.vector.tensor_mul(sc2[:], s[:], c2[:])
                # alternate the 4th product between DVE and GPSIMD to balance
                eng = nc.vector if blk % 2 == 0 else nc.gpsimd
                eng.tensor_mul(sc3[:], sc[:], c2[:])
                nc.gpsimd.tensor_mul(sc4[:], s[:], c4[:])
                nc.gpsimd.tensor_mul(c5[:], c[:], c4[:])

                forder = [s, c, sc, c2, sc2, c3, sc3, c4, sc4, c5]

                out_sb = opool.tile([128, 2 * BLKCOL], F32, tag="o")
                # 8 concurrent MMs per j: 4 row-groups x both col-halves of
                # the PE array (half 1 uses the complementary col-group so
                # all 16 subarrays stay busy).
                # one 2-bank PSUM tile per half: bank a at columns a*512
                ps2 = {
                    half: ppool.tile(
                        [128, 2 * HALF], F32, name=f"ps{half}", tag=f"ps{half}"
                    )
                    for half in range(NHALF)
                }
                ps = {
                    (half, a): ps2[half][:, a * HALF : (a + 1) * HALF]
                    for half in range(NHALF)
                    for a in range(2)
                }
                for j in range(NJ):
                    for g in range(NGRP):
                        a = g // 2
                        for half in range(NHALF):
                            bcol = (g + half) % 2
                            hs = slice(half * HALF, (half + 1) * HALF)
                            nc.tensor.matmul(
                                ps[(half, a)][64 * bcol : 64 * bcol + 64, :],
                                w_sb[32 * g : 32 * g + 32, j, 64 * bcol : 64 * bcol + 64],
                                forder[j][32 * g : 32 * g + 32, hs],
                                start=(j == 0),
                                stop=(j == NJ - 1),
                                tile_position=(32 * g, 64 * bcol),
                            )
                for half in range(NHALF):
                    dst = out_sb[:, 2 * half * HALF : 2 * (half + 1) * HALF]
                    nc.scalar.activation(
                        dst, ps2[half][:], AF.Identity, bias=bias_sb[:], scale=1.0
                    )
                nc.sync.dma_start(
                    out=out_d[:, blk * 2 * BLKCOL : (blk + 1) * 2 * BLKCOL],
                    in_=out_sb[:],
                )

    nc.compile()
    return nc


def _get_program():
    global _PROG
    if _PROG is None:
        _PROG = _build_program()
    return _PROG


def _prep_inputs(x, W, b):
    """Host-side: shard + layout x, fold basis transform into W."""
    x = np.asarray(x)
    W64 = np.asarray(W, dtype=np.float64)
    b64 = np.asarray(b, dtype=np.float64)

    # W2[f, m, h] = sum_i A[i, m] * W[f, i, h]
    W2 = np.einsum("im,fih->fmh", _A, W64)
    bias = W2[:, 0, :].sum(axis=0) + b64.sum(axis=0)          # [H]
    bias_col = np.tile(bias, 2).astype(np.float32)[:, None]    # [128, 1]
    bias_col = np.ascontiguousarray(bias_col)

    # per-j matmul weights, replicated 4x along K-groups and 2x along H
    w_list = [np.tile(W2[:, 1 + j, :], (NGRP, 2)) for j in range(NJ)]  # [128,128]
    w_flat = (
        np.stack(w_list, axis=0).transpose(1, 0, 2).reshape(128, NJ * 128)
    ).astype(np.float32)
    w_flat = np.ascontiguousarray(w_flat)

    xt = x.reshape(B * T, F).astype(np.float64)
    xr = np.mod(xt + np.pi, 2.0 * np.pi) - np.pi               # [-pi, pi)
    xr = xr.astype(np.float32)

    xs = []
    for cid in range(NCORES):
        xc = xr[cid * TOKPC : (cid + 1) * TOKPC]               # [16384, 32]
        xc = xc.reshape(NGRP, GTOK, F).transpose(0, 2, 1).reshape(128, GTOK)
        xs.append(np.ascontiguousarray(xc))
    return xs, w_flat, bias_col


def _decode_out(outc):
    """[128, 8192] device layout -> [TOKPC, H].

    Rows are 64*bcol + h; for half 0 group g sits at bcol = g%2, for half 1
    at bcol = (g+1)%2 (complementary PE col-group packing)."""
    arr = outc.reshape(2, H, NBLK, NHALF, 2, HALF)   # [bcol, h, blk, half, a, cc]
    h0 = arr[:, :, :, 0]                             # [b, h, blk, a, cc], g = 2a+b
    h1 = arr[::-1, :, :, 1]                          # b-index == g%2 after flip
    both = np.stack([h0, h1], axis=3)                # [b, h, blk, half, a, cc]
    both = both.transpose(4, 0, 2, 3, 5, 1)          # [a, b, blk, half, cc, h]
    return both.reshape(TOKPC, H)


LAST_RESULT = None


def kernel(x, W, b, trace=False, tmpdir=None):
    nc = _get_program()
    xs, w_flat, bias_col = _prep_inputs(x, W, b)
    in_maps = [
        {"x": xs[cid], "w": w_flat, "bias": bias_col} for cid in range(NCORES)
    ]
    res = run_bass_kernel_spmd(
        nc, in_maps, list(range(NCORES)), trace=trace, tmpdir=tmpdir
    )
    global LAST_RESULT
    LAST_RESULT = res
    out = np.empty((B * T, H), dtype=np.float32)
    for cid in range(NCORES):
        out[cid * TOKPC : (cid + 1) * TOKPC] = _decode_out(res.results[cid]["out"])
    return out.reshape(B, T, H)
